# revision 8
# baseline (speedup 1.0000x reference)
"""ComplexLSTM Trainium2 kernel.

Problem: x [2, 64, 128, 1024] (real/imag, B, I, T) -> out [2, 64, 256, 1024].
Four real LSTM applications: lstm_r(x_real), lstm_r(x_imag), lstm_i(x_real),
lstm_i(x_imag); combined as L_r = r(xr) - i(xim), L_i = r(xim) + i(xr).

Sharding: 2 weight-sets x 128 sequences each = 256 independent sequences.
8 cores x 32 sequences (cores 0-3: r-weights, cores 4-7: i-weights).

Device layout (fully transposed state, weights-stationary matmuls):
  PSUM is one [128, 8, 2, 8, 32] f32 tile: bank m = gate block m (order
  [g,g,i,i,f,f,o,o]), split in two half-chunks of 8 steps x 32 batch.
  The x-projection + bias for each half-chunk are pre-accumulated into PSUM
  off the critical path (8 fat Wih matmuls free=256 + 8 indicator bias
  matmuls); the per-step work on PE is only the 16 recurrent Whh matmuls
  (lhsT = WhhT tiles bf16, rhs = h'.T slice) accumulating on top.
  Tail per step: one sigmoid over all 8 blocks (g-gate rows pre-scaled 2x on
  host so tanh(g) = 2*sig(2g)-1), then fused DVE ops on [128,2,32]:
    v_q = (s_g - 0.5) * s_i            (= i*tanh(g) / 2)
    u   = s_f * c2_prev
    c2  = 4*v_q + u                    (c2 = 2*c)
    s_c = sigmoid(c2)                  (= (tanh(c)+1)/2)
    h'  = (s_c - 0.5) * s_o            (= h / 2)
  h' written bf16 into a T-chunk history buffer that doubles as the matmul
  rhs for the next step; Whh is pre-scaled 2x to compensate h'=h/2 and the
  host combine multiplies the final output by 2.
"""

import numpy as np
import ml_dtypes
from contextlib import ExitStack

import concourse.bass as bass
import concourse.bacc as bacc
import concourse.tile as tile
from concourse import mybir
from concourse.bass_utils import run_bass_kernel_spmd

BF16 = mybir.dt.bfloat16
F32 = mybir.dt.float32
AF = mybir.ActivationFunctionType
OP = mybir.AluOpType

B, I, T_FULL, H = 64, 128, 1024, 256
NB = 32          # batch (sequences) per core
NCORES = 8
SC = 8           # steps per PSUM half-chunk
TC = 128         # history chunk (steps per output DMA)
XC = 64          # x input chunk (steps per input DMA)

_cache = {}


def build(T):
    nc = bacc.Bacc("TRN2", target_bir_lowering=False, debug=False)

    tc_hist = max(1, min(TC, T))
    xc = max(1, min(XC, T))
    assert T % tc_hist == 0 and T % xc == 0 and T % SC == 0

    xT_d = nc.declare_dram_parameter("xT", [128, T, NB], BF16, isOutput=False)
    whhT_d = nc.declare_dram_parameter("whhT", [128, 2, 8, 128], BF16, isOutput=False)
    wihT_d = nc.declare_dram_parameter("wihT", [128, 8, 128], BF16, isOutput=False)
    biasK_d = nc.declare_dram_parameter("biasK", [8, 128], BF16, isOutput=False)
    ind_d = nc.declare_dram_parameter("ind", [8, 8 * SC * NB], BF16, isOutput=False)
    hist_d = nc.declare_dram_parameter("hist", [128, 2, NB, T], BF16, isOutput=True)

    with tile.TileContext(nc) as tc, ExitStack() as ctx:
        consts = ctx.enter_context(tc.tile_pool(name="consts", bufs=1))
        xin = ctx.enter_context(tc.tile_pool(name="xin", bufs=2))
        hpool = ctx.enter_context(tc.tile_pool(name="hist", bufs=2))
        psum = ctx.enter_context(tc.tile_pool(name="psum", bufs=1, space="PSUM"))
        sml = ctx.enter_context(tc.tile_pool(name="small", bufs=3))
        cpool = ctx.enter_context(tc.tile_pool(name="cpool", bufs=3))

        WHH = consts.tile([128, 2, 8, 128], BF16)
        nc.sync.dma_start(WHH[:], whhT_d[:])
        WIH = consts.tile([128, 8, 128], BF16)
        nc.sync.dma_start(WIH[:], wihT_d[:])
        BIASK = consts.tile([8, 128], BF16)
        nc.sync.dma_start(BIASK[:], biasK_d[:])
        IND = consts.tile([8, 8 * SC * NB], BF16)
        nc.sync.dma_start(IND[:], ind_d[:])
        ZH = consts.tile([128, 2, NB], BF16)
        nc.vector.memset(ZH[:], 0.0)
        ZC = consts.tile([128, 2, NB], F32)
        nc.vector.memset(ZC[:], 0.0)

        # All of PSUM: [partition, chunk-parity, bank-in-group, block-in-bank,
        # step-in-chunk, batch]. Chunk parity p uses banks 4p..4p+3; bank
        # q holds gate blocks 2q and 2q+1. start=True (bank-granular reset,
        # ZERO_REGION=2KB) is issued only on the first matmul into each bank
        # per chunk.
        PS = psum.tile([128, 2, 4, 2, SC, NB], F32)

        HIST = None
        c_prev = None
        h_prev = None  # AP into HIST for h'.T(t-1)
        nxc = T // xc
        xbufs = {}  # x chunk index -> SBUF tile (bufs=2 pool keeps 2 live)

        def load_xchunk(c):
            if c < nxc and c not in xbufs:
                xb = xin.tile([128, xc, NB], BF16, tag="xbuf")
                nc.sync.dma_start(xb[:], xT_d[:, c * xc:(c + 1) * xc, :])
                xbufs[c] = xb

        def fill_half(t0):
            """Emit xproj + bias matmuls for steps t0..t0+SC-1 (into parity
            (t0//SC)%2). Off the critical path: runs on PE during tails."""
            hf = (t0 // SC) % 2
            xb = xbufs[t0 // xc]
            xsl = xb[:, t0 % xc:t0 % xc + SC, :]
            for q in range(4):
                for r in range(2):
                    m = 2 * q + r
                    dst = PS[:, hf, q, r, :, :]
                    # rhs covers SC steps x NB batch = 256 free elems
                    nc.tensor.matmul(
                        dst, WIH[:, m, :], xsl,
                        start=(r == 0), stop=False, skip_group_check=True,
                    )
                    nc.tensor.matmul(
                        dst, BIASK[:], IND[:, m * SC * NB:(m + 1) * SC * NB],
                        start=False, stop=False, skip_group_check=True,
                    )

        for t in range(T):
            if t % xc == 0:
                load_xchunk(t // xc)
                load_xchunk(t // xc + 1)  # prefetch: fills read ahead of t
                xbufs.pop(t // xc - 2, None)
                if t == 0:
                    fill_half(0)
                    if T > SC:
                        fill_half(SC)
            th = t % tc_hist
            if th == 0:
                HIST = hpool.tile([128, 2, NB, tc_hist], BF16, tag="hist")

            hf = (t // SC) % 2
            s8 = t % SC
            g_ps = PS[:, hf, :, :, s8, :]       # [128, 4, 2, 32] = 8 blocks
            h_rhs = h_prev if t > 0 else ZH[:]
            for m in range(8):
                for k in range(2):
                    nc.tensor.matmul(
                        PS[:, hf, m // 2, m % 2, s8, :],
                        WHH[:, k, m, :], h_rhs[:, k, :],
                        start=False, stop=(k == 1), skip_group_check=True,
                    )

            # single sigmoid over all 8 blocks: [g,g,i,i,f,f,o,o]
            s = sml.tile([128, 8, NB], F32, tag="s")
            nc.scalar.activation(s[:], g_ps, AF.Sigmoid)

            vq = sml.tile([128, 2, NB], F32, tag="vq")
            nc.vector.scalar_tensor_tensor(
                vq[:], s[:, 0:2, :], 0.5, s[:, 2:4, :], OP.subtract, OP.mult)
            u = sml.tile([128, 2, NB], F32, tag="u")
            cp = c_prev if t > 0 else ZC[:]
            nc.vector.tensor_tensor(u[:], s[:, 4:6, :], cp, OP.mult)
            c_new = cpool.tile([128, 2, NB], F32, tag="c")
            nc.vector.scalar_tensor_tensor(
                c_new[:], vq[:], 4.0, u[:], OP.mult, OP.add)
            sc_t = sml.tile([128, 2, NB], F32, tag="sc")
            nc.scalar.activation(sc_t[:], c_new[:], AF.Sigmoid)
            h_slot = HIST[:, :, :, th]
            nc.vector.scalar_tensor_tensor(
                h_slot, sc_t[:], 0.5, s[:, 6:8, :], OP.subtract, OP.mult)

            c_prev = c_new[:]
            h_prev = HIST[:, :, :, th]

            # refill the half-chunk we just finished consuming
            if s8 == SC - 1 and t + 1 + SC < T:
                fill_half(t + 1 + SC)

            if th == tc_hist - 1:
                t0 = t - (tc_hist - 1)
                nc.sync.dma_start(hist_d[:, :, :, t0:t0 + tc_hist], HIST[:])
    nc.compile()
    return nc


def _get_nc(T):
    if T not in _cache:
        _cache[T] = build(T)
    return _cache[T]


def _prep_core_inputs(x, Wih, Whh, bih, bhh, T):
    """Per weight-set host prep. Returns (shared weight arrays, xT per 4 cores).

    Gate order permuted torch [i,f,g,o] -> [g,i,f,o]; g rows scaled 2x
    (tanh-as-sigmoid trick); all Whh columns scaled 2x (h stored halved)."""
    perm = np.concatenate([np.arange(512, 768), np.arange(0, 256),
                           np.arange(256, 512), np.arange(768, 1024)])
    rowscale = np.ones((1024, 1), np.float32)
    rowscale[0:256] = 2.0   # g rows (after permutation)
    Wihp = np.asarray(Wih)[perm] * rowscale            # [1024, 128]
    Whhp = np.asarray(Whh)[perm] * (2.0 * rowscale)    # [1024, 256]
    biasp = ((np.asarray(bih) + np.asarray(bhh))[perm] * rowscale[:, 0])

    whhT = Whhp.reshape(8, 128, 2, 128).transpose(3, 2, 0, 1)  # [p,k,m,j]
    wihT = Wihp.reshape(8, 128, 128).transpose(2, 0, 1)        # [p,m,j]
    biasK = biasp.reshape(8, 128)
    whhT = whhT.astype(ml_dtypes.bfloat16)
    wihT = wihT.astype(ml_dtypes.bfloat16)
    biasK = biasK.astype(ml_dtypes.bfloat16)

    # batch-128 for this weight set: seqs 0-63 = x_real (x[0]), 64-127 = x_imag
    # x: [2, B, I, T]; per seq [I, T] slice. xT per core: [128, T, 32]
    xTs = []
    xall = np.concatenate([np.asarray(x)[0], np.asarray(x)[1]], axis=0)  # [128, I, T]
    for g in range(4):
        sl = xall[32 * g:32 * g + 32]             # [32, I, T]
        xT = sl.transpose(1, 2, 0)[:, :T, :]      # [I, T, 32]
        xTs.append(np.ascontiguousarray(xT).astype(ml_dtypes.bfloat16))
    return whhT, wihT, biasK, xTs


def _run(x, Wih_r, Whh_r, bih_r, bhh_r, Wih_i, Whh_i, bih_i, bhh_i, T,
         trace=False, tmpdir=None):
    nc = _get_nc(T)
    ind = np.kron(np.eye(8), np.ones((1, SC * NB))).astype(ml_dtypes.bfloat16)

    whhT_r, wihT_r, biasK_r, xTs_r = _prep_core_inputs(x, Wih_r, Whh_r, bih_r, bhh_r, T)
    whhT_i, wihT_i, biasK_i, _ = _prep_core_inputs(x, Wih_i, Whh_i, bih_i, bhh_i, T)
    xTs_i = xTs_r  # same input data for both weight sets

    in_maps = []
    for core in range(NCORES):
        ws = core // 4
        g = core % 4
        whhT, wihT, biasK = (whhT_r, wihT_r, biasK_r) if ws == 0 else (whhT_i, wihT_i, biasK_i)
        xT = (xTs_r if ws == 0 else xTs_i)[g]
        in_maps.append({
            "xT": xT, "whhT": whhT, "wihT": wihT, "biasK": biasK, "ind": ind,
        })
    res = run_bass_kernel_spmd(nc, in_maps, core_ids=list(range(NCORES)),
                               trace=trace, tmpdir=tmpdir)
    results = res.results

    # reassemble: hist [128, 2, 32, T] (h/2 in bf16) -> [H=256, 32, T] per core
    def hmat(ws):
        parts = []
        for g in range(4):
            h = results[4 * ws + g]["hist"].astype(np.float32)
            parts.append(h.transpose(1, 0, 2, 3).reshape(256, NB, T))
        return np.concatenate(parts, axis=1)  # [256, 128, T]

    Hr = hmat(0)
    Hi = hmat(1)
    L_r = (Hr[:, 0:64] - Hi[:, 64:128]) * 2.0   # [256, 64, T]; 2x undoes h/2
    L_i = (Hr[:, 64:128] + Hi[:, 0:64]) * 2.0
    out = np.stack([L_r.transpose(1, 0, 2), L_i.transpose(1, 0, 2)], axis=0)
    return np.ascontiguousarray(out.astype(np.float32)), res


def kernel(x, Wih_r, Whh_r, bih_r, bhh_r, Wih_i, Whh_i, bih_i, bhh_i):
    out, _ = _run(x, Wih_r, Whh_r, bih_r, bhh_r,
                  Wih_i, Whh_i, bih_i, bhh_i, T_FULL)
    return out


# revision 13
# speedup vs baseline: 1.2383x; 1.2383x over previous
"""ComplexLSTM Trainium2 kernel.

Problem: x [2, 64, 128, 1024] (real/imag, B, I, T) -> out [2, 64, 256, 1024].
Four real LSTM applications: lstm_r(x_real), lstm_r(x_imag), lstm_i(x_real),
lstm_i(x_imag); combined as L_r = r(xr) - i(xim), L_i = r(xim) + i(xr).

Sharding: 2 weight-sets x 128 sequences each = 256 independent sequences.
8 cores x 32 sequences (cores 0-3: r-weights, cores 4-7: i-weights).

Device layout (fully transposed state, weights-stationary matmuls):
  PSUM is one [128, 8, 2, 8, 32] f32 tile: bank m = gate block m (order
  [g,g,i,i,f,f,o,o]), split in two half-chunks of 8 steps x 32 batch.
  The x-projection + bias for each half-chunk are pre-accumulated into PSUM
  off the critical path (8 fat Wih matmuls free=256 + 8 indicator bias
  matmuls); the per-step work on PE is only the 16 recurrent Whh matmuls
  (lhsT = WhhT tiles bf16, rhs = h'.T slice) accumulating on top.
  Tail per step: one sigmoid over all 8 blocks (g-gate rows pre-scaled 2x on
  host so tanh(g) = 2*sig(2g)-1), then fused DVE ops on [128,2,32]:
    v_q = (s_g - 0.5) * s_i            (= i*tanh(g) / 2)
    u   = s_f * c2_prev
    c2  = 4*v_q + u                    (c2 = 2*c)
    s_c = sigmoid(c2)                  (= (tanh(c)+1)/2)
    h'  = (s_c - 0.5) * s_o            (= h / 2)
  h' written bf16 into a T-chunk history buffer that doubles as the matmul
  rhs for the next step; Whh is pre-scaled 2x to compensate h'=h/2 and the
  host combine multiplies the final output by 2.
"""

import numpy as np
import ml_dtypes
from contextlib import ExitStack

import concourse.bass as bass
import concourse.bacc as bacc
import concourse.tile as tile
from concourse import mybir
from concourse.bass_utils import run_bass_kernel_spmd

BF16 = mybir.dt.bfloat16
F32 = mybir.dt.float32
AF = mybir.ActivationFunctionType
OP = mybir.AluOpType

B, I, T_FULL, H = 64, 128, 1024, 256
NB = 32          # batch (sequences) per core
NCORES = 8
SC = 8           # steps per PSUM half-chunk
TC = 128         # history chunk (steps per output DMA)
XC = 64          # x input chunk (steps per input DMA)

_cache = {}


def build(T):
    nc = bacc.Bacc("TRN2", target_bir_lowering=False, debug=False)

    tc_hist = max(1, min(TC, T))
    xc = max(1, min(XC, T))
    assert T % tc_hist == 0 and T % xc == 0 and T % SC == 0

    xT_d = nc.declare_dram_parameter("xT", [128, T, NB], BF16, isOutput=False)
    whhT_d = nc.declare_dram_parameter("whhT", [128, 2, 8, 128], BF16, isOutput=False)
    wihT_d = nc.declare_dram_parameter("wihT", [128, 8, 128], BF16, isOutput=False)
    biasK_d = nc.declare_dram_parameter("biasK", [8, 128], BF16, isOutput=False)
    ind_d = nc.declare_dram_parameter("ind", [8, 8 * SC * NB], BF16, isOutput=False)
    hist_d = nc.declare_dram_parameter("hist", [128, T, 2, NB], BF16, isOutput=True)

    with tile.TileContext(nc) as tc, ExitStack() as ctx:
        consts = ctx.enter_context(tc.tile_pool(name="consts", bufs=1))
        xin = ctx.enter_context(tc.tile_pool(name="xin", bufs=2))
        hpool = ctx.enter_context(tc.tile_pool(name="hist", bufs=2))
        psum = ctx.enter_context(tc.tile_pool(name="psum", bufs=1, space="PSUM"))
        sml = ctx.enter_context(tc.tile_pool(name="small", bufs=3))
        cpool = ctx.enter_context(tc.tile_pool(name="cpool", bufs=3))

        WHH = consts.tile([128, 2, 8, 128], BF16)
        nc.sync.dma_start(WHH[:], whhT_d[:])
        WIH = consts.tile([128, 8, 128], BF16)
        nc.sync.dma_start(WIH[:], wihT_d[:])
        BIASK = consts.tile([8, 128], BF16)
        nc.sync.dma_start(BIASK[:], biasK_d[:])
        IND = consts.tile([8, 8 * SC * NB], BF16)
        nc.sync.dma_start(IND[:], ind_d[:])
        ZH = consts.tile([128, 2, NB], BF16)
        nc.vector.memset(ZH[:], 0.0)
        ZC = consts.tile([128, 2, NB], F32)
        nc.vector.memset(ZC[:], 0.0)

        # All of PSUM: [partition, chunk-parity, bank-in-group, block-in-bank,
        # step-in-chunk, batch]. Chunk parity p uses banks 4p..4p+3; bank
        # q holds gate blocks 2q and 2q+1. start=True (bank-granular reset,
        # ZERO_REGION=2KB) is issued only on the first matmul into each bank
        # per chunk.
        PS = psum.tile([128, 2, 4, 2, SC, NB], F32)

        HIST = None
        c_prev = None
        h_prev = None  # AP into HIST for h'.T(t-1)
        nxc = T // xc
        xbufs = {}  # x chunk index -> SBUF tile (bufs=2 pool keeps 2 live)

        def load_xchunk(c):
            if c < nxc and c not in xbufs:
                xb = xin.tile([128, xc, NB], BF16, tag="xbuf")
                nc.sync.dma_start(xb[:], xT_d[:, c * xc:(c + 1) * xc, :])
                xbufs[c] = xb

        def fill_block(t0, m):
            """Emit xproj + bias matmul for gate block m of the chunk at
            steps t0..t0+SC-1 (parity (t0//SC)%2). Off the critical path:
            runs on PE during tails. Blocks must be emitted in order
            (start=True on the even block resets the whole bank)."""
            hf = (t0 // SC) % 2
            xb = xbufs[t0 // xc]
            xsl = xb[:, t0 % xc:t0 % xc + SC, :]
            q, r = m // 2, m % 2
            dst = PS[:, hf, q, r, :, :]
            # rhs covers SC steps x NB batch = 256 free elems
            nc.tensor.matmul(
                dst, WIH[:, m, :], xsl,
                start=(r == 0), stop=False, skip_group_check=True,
            )
            nc.tensor.matmul(
                dst, BIASK[:], IND[:, m * SC * NB:(m + 1) * SC * NB],
                start=False, stop=False, skip_group_check=True,
            )

        def fill_half(t0):
            for m in range(8):
                fill_block(t0, m)

        for t in range(T):
            if t % xc == 0:
                load_xchunk(t // xc)
                load_xchunk(t // xc + 1)  # prefetch: fills read ahead of t
                xbufs.pop(t // xc - 2, None)
                if t == 0:
                    fill_half(0)
                    if T > SC:
                        fill_half(SC)
            th = t % tc_hist
            if th == 0:
                HIST = hpool.tile([128, tc_hist, 2, NB], BF16, tag="hist")

            hf = (t // SC) % 2
            s8 = t % SC
            g_ps = PS[:, hf, :, :, s8, :]       # [128, 4, 2, 32] = 8 blocks
            h_rhs = h_prev if t > 0 else ZH[:]
            for m in range(8):
                for k in range(2):
                    nc.tensor.matmul(
                        PS[:, hf, m // 2, m % 2, s8, :],
                        WHH[:, k, m, :], h_rhs[:, k, :],
                        start=False, stop=(k == 1), skip_group_check=True,
                    )

            # single sigmoid over all 8 blocks: [g,g,i,i,f,f,o,o]
            s = sml.tile([128, 8, NB], F32, tag="s")
            nc.scalar.activation(s[:], g_ps, AF.Sigmoid)

            vq = sml.tile([128, 2, NB], F32, tag="vq")
            nc.vector.scalar_tensor_tensor(
                vq[:], s[:, 0:2, :], 0.5, s[:, 2:4, :], OP.subtract, OP.mult)
            u = sml.tile([128, 2, NB], F32, tag="u")
            cp = c_prev if t > 0 else ZC[:]
            nc.vector.tensor_tensor(u[:], s[:, 4:6, :], cp, OP.mult)
            c_new = cpool.tile([128, 2, NB], F32, tag="c")
            nc.vector.scalar_tensor_tensor(
                c_new[:], vq[:], 4.0, u[:], OP.mult, OP.add)
            sc_t = sml.tile([128, 2, NB], F32, tag="sc")
            nc.scalar.activation(sc_t[:], c_new[:], AF.Sigmoid)
            h_slot = HIST[:, th, :, :]
            nc.vector.scalar_tensor_tensor(
                h_slot, sc_t[:], 0.5, s[:, 6:8, :], OP.subtract, OP.mult)

            c_prev = c_new[:]
            h_prev = HIST[:, th, :, :]

            # spread the next chunk's fill: one gate block per step, emitted
            # during chunk c for chunk c+1 (banks freed at end of chunk c-1)
            if t >= SC and (t // SC + 2) * SC <= T:
                fill_block((t // SC + 1) * SC, s8)

            if th == tc_hist - 1:
                t0 = t - (tc_hist - 1)
                nc.sync.dma_start(hist_d[:, t0:t0 + tc_hist, :, :], HIST[:])
    nc.compile()
    return nc


def _get_nc(T):
    if T not in _cache:
        _cache[T] = build(T)
    return _cache[T]


def _prep_core_inputs(x, Wih, Whh, bih, bhh, T):
    """Per weight-set host prep. Returns (shared weight arrays, xT per 4 cores).

    Gate order permuted torch [i,f,g,o] -> [g,i,f,o]; g rows scaled 2x
    (tanh-as-sigmoid trick); all Whh columns scaled 2x (h stored halved)."""
    perm = np.concatenate([np.arange(512, 768), np.arange(0, 256),
                           np.arange(256, 512), np.arange(768, 1024)])
    rowscale = np.ones((1024, 1), np.float32)
    rowscale[0:256] = 2.0   # g rows (after permutation)
    Wihp = np.asarray(Wih)[perm] * rowscale            # [1024, 128]
    Whhp = np.asarray(Whh)[perm] * (2.0 * rowscale)    # [1024, 256]
    biasp = ((np.asarray(bih) + np.asarray(bhh))[perm] * rowscale[:, 0])

    whhT = Whhp.reshape(8, 128, 2, 128).transpose(3, 2, 0, 1)  # [p,k,m,j]
    wihT = Wihp.reshape(8, 128, 128).transpose(2, 0, 1)        # [p,m,j]
    biasK = biasp.reshape(8, 128)
    whhT = whhT.astype(ml_dtypes.bfloat16)
    wihT = wihT.astype(ml_dtypes.bfloat16)
    biasK = biasK.astype(ml_dtypes.bfloat16)

    # batch-128 for this weight set: seqs 0-63 = x_real (x[0]), 64-127 = x_imag
    # x: [2, B, I, T]; per seq [I, T] slice. xT per core: [128, T, 32]
    xTs = []
    xall = np.concatenate([np.asarray(x)[0], np.asarray(x)[1]], axis=0)  # [128, I, T]
    for g in range(4):
        sl = xall[32 * g:32 * g + 32]             # [32, I, T]
        xT = sl.transpose(1, 2, 0)[:, :T, :]      # [I, T, 32]
        xTs.append(np.ascontiguousarray(xT).astype(ml_dtypes.bfloat16))
    return whhT, wihT, biasK, xTs


def _run(x, Wih_r, Whh_r, bih_r, bhh_r, Wih_i, Whh_i, bih_i, bhh_i, T,
         trace=False, tmpdir=None):
    nc = _get_nc(T)
    ind = np.kron(np.eye(8), np.ones((1, SC * NB))).astype(ml_dtypes.bfloat16)

    whhT_r, wihT_r, biasK_r, xTs_r = _prep_core_inputs(x, Wih_r, Whh_r, bih_r, bhh_r, T)
    whhT_i, wihT_i, biasK_i, _ = _prep_core_inputs(x, Wih_i, Whh_i, bih_i, bhh_i, T)
    xTs_i = xTs_r  # same input data for both weight sets

    in_maps = []
    for core in range(NCORES):
        ws = core // 4
        g = core % 4
        whhT, wihT, biasK = (whhT_r, wihT_r, biasK_r) if ws == 0 else (whhT_i, wihT_i, biasK_i)
        xT = (xTs_r if ws == 0 else xTs_i)[g]
        in_maps.append({
            "xT": xT, "whhT": whhT, "wihT": wihT, "biasK": biasK, "ind": ind,
        })
    res = run_bass_kernel_spmd(nc, in_maps, core_ids=list(range(NCORES)),
                               trace=trace, tmpdir=tmpdir)
    results = res.results

    # reassemble: hist [128, 2, 32, T] (h/2 in bf16) -> [H=256, 32, T] per core
    def hmat(ws):
        parts = []
        for g in range(4):
            h = results[4 * ws + g]["hist"].astype(np.float32)
            parts.append(h.transpose(2, 0, 3, 1).reshape(256, NB, T))
        return np.concatenate(parts, axis=1)  # [256, 128, T]

    Hr = hmat(0)
    Hi = hmat(1)
    L_r = (Hr[:, 0:64] - Hi[:, 64:128]) * 2.0   # [256, 64, T]; 2x undoes h/2
    L_i = (Hr[:, 64:128] + Hi[:, 0:64]) * 2.0
    out = np.stack([L_r.transpose(1, 0, 2), L_i.transpose(1, 0, 2)], axis=0)
    return np.ascontiguousarray(out.astype(np.float32)), res


def kernel(x, Wih_r, Whh_r, bih_r, bhh_r, Wih_i, Whh_i, bih_i, bhh_i):
    out, _ = _run(x, Wih_r, Whh_r, bih_r, bhh_r,
                  Wih_i, Whh_i, bih_i, bhh_i, T_FULL)
    return out


# revision 17
# speedup vs baseline: 1.2605x; 1.0179x over previous
"""ComplexLSTM Trainium2 kernel.

Problem: x [2, 64, 128, 1024] (real/imag, B, I, T) -> out [2, 64, 256, 1024].
Four real LSTM applications: lstm_r(x_real), lstm_r(x_imag), lstm_i(x_real),
lstm_i(x_imag); combined as L_r = r(xr) - i(xim), L_i = r(xim) + i(xr).

Sharding: 2 weight-sets x 128 sequences each = 256 independent sequences.
8 cores x 32 sequences (cores 0-3: r-weights, cores 4-7: i-weights).

Device layout (fully transposed state, weights-stationary matmuls):
  PSUM is one [128, 8, 2, 8, 32] f32 tile: bank m = gate block m (order
  [g,g,i,i,f,f,o,o]), split in two half-chunks of 8 steps x 32 batch.
  The x-projection + bias for each half-chunk are pre-accumulated into PSUM
  off the critical path (8 fat Wih matmuls free=256 + 8 indicator bias
  matmuls); the per-step work on PE is only the 16 recurrent Whh matmuls
  (lhsT = WhhT tiles bf16, rhs = h'.T slice) accumulating on top.
  Tail per step: one sigmoid over all 8 blocks (g-gate rows pre-scaled 2x on
  host so tanh(g) = 2*sig(2g)-1), then fused DVE ops on [128,2,32]:
    v_q = (s_g - 0.5) * s_i            (= i*tanh(g) / 2)
    u   = s_f * c2_prev
    c2  = 4*v_q + u                    (c2 = 2*c)
    s_c = sigmoid(c2)                  (= (tanh(c)+1)/2)
    h'  = (s_c - 0.5) * s_o            (= h / 2)
  h' written bf16 into a T-chunk history buffer that doubles as the matmul
  rhs for the next step; Whh is pre-scaled 2x to compensate h'=h/2 and the
  host combine multiplies the final output by 2.
"""

import numpy as np
import ml_dtypes
from contextlib import ExitStack

import concourse.bass as bass
import concourse.bacc as bacc
import concourse.tile as tile
from concourse import mybir
from concourse.bass_utils import run_bass_kernel_spmd

BF16 = mybir.dt.bfloat16
F32 = mybir.dt.float32
AF = mybir.ActivationFunctionType
OP = mybir.AluOpType

B, I, T_FULL, H = 64, 128, 1024, 256
NB = 32          # batch (sequences) per core
NCORES = 8
SC = 8           # steps per PSUM half-chunk
TC = 128         # history chunk (steps per output DMA)
XC = 64          # x input chunk (steps per input DMA)

_cache = {}


def build(T):
    nc = bacc.Bacc("TRN2", target_bir_lowering=False, debug=False)

    tc_hist = max(1, min(TC, T))
    xc = max(1, min(XC, T))
    assert T % tc_hist == 0 and T % xc == 0 and T % SC == 0

    xT_d = nc.declare_dram_parameter("xT", [128, T, NB], BF16, isOutput=False)
    whhT_d = nc.declare_dram_parameter("whhT", [128, 2, 8, 128], BF16, isOutput=False)
    wihT_d = nc.declare_dram_parameter("wihT", [128, 8, 128], BF16, isOutput=False)
    biasK_d = nc.declare_dram_parameter("biasK", [8, 128], BF16, isOutput=False)
    ind_d = nc.declare_dram_parameter("ind", [8, 8 * SC * NB], BF16, isOutput=False)
    hist_d = nc.declare_dram_parameter("hist", [128, T, 2, NB], BF16, isOutput=True)

    with tile.TileContext(nc) as tc, ExitStack() as ctx:
        consts = ctx.enter_context(tc.tile_pool(name="consts", bufs=1))
        xin = ctx.enter_context(tc.tile_pool(name="xin", bufs=2))
        hpool = ctx.enter_context(tc.tile_pool(name="hist", bufs=2))
        psum = ctx.enter_context(tc.tile_pool(name="psum", bufs=1, space="PSUM"))
        sml = ctx.enter_context(tc.tile_pool(name="small", bufs=3))
        cpool = ctx.enter_context(tc.tile_pool(name="cpool", bufs=3))

        WHH = consts.tile([128, 2, 8, 128], BF16)
        nc.sync.dma_start(WHH[:], whhT_d[:])
        WIH = consts.tile([128, 8, 128], BF16)
        nc.sync.dma_start(WIH[:], wihT_d[:])
        BIASK = consts.tile([8, 128], BF16)
        nc.sync.dma_start(BIASK[:], biasK_d[:])
        IND = consts.tile([8, 8 * SC * NB], BF16)
        nc.sync.dma_start(IND[:], ind_d[:])
        ZH = consts.tile([128, 2, NB], BF16)
        nc.vector.memset(ZH[:], 0.0)
        ZC = consts.tile([128, 2, NB], F32)
        nc.vector.memset(ZC[:], 0.0)

        # All of PSUM: [partition, bank-in-group, block-in-bank,
        # step-in-chunk, batch] x 2 parities. Chunk parity p uses banks
        # 4p..4p+3; bank q holds gate blocks 2q and 2q+1. start=True
        # (bank-granular reset, ZERO_REGION=2KB) is issued only on the first
        # matmul into each bank per chunk. Two separate tiles so the Tile
        # framework's per-tile dependency tracking doesn't serialize the
        # next chunk's fill behind the current chunk's sigmoid reads.
        PSA = psum.tile([128, 4, 2, SC, NB], F32, tag="psA")
        PSB = psum.tile([128, 4, 2, SC, NB], F32, tag="psB")
        PS2 = [PSA, PSB]

        HIST = None
        c_prev = None
        h_prev = None  # AP into HIST for h'.T(t-1)
        nxc = T // xc
        xbufs = {}  # x chunk index -> SBUF tile (bufs=2 pool keeps 2 live)

        def load_xchunk(c):
            if c < nxc and c not in xbufs:
                xb = xin.tile([128, xc, NB], BF16, tag="xbuf")
                nc.sync.dma_start(xb[:], xT_d[:, c * xc:(c + 1) * xc, :])
                xbufs[c] = xb

        def fill_block(t0, m):
            """Emit xproj + bias matmul for gate block m of the chunk at
            steps t0..t0+SC-1 (parity (t0//SC)%2). Off the critical path:
            runs on PE during tails. Blocks must be emitted in order
            (start=True on the even block resets the whole bank)."""
            hf = (t0 // SC) % 2
            xb = xbufs[t0 // xc]
            xsl = xb[:, t0 % xc:t0 % xc + SC, :]
            q, r = m // 2, m % 2
            dst = PS2[hf][:, q, r, :, :]
            # rhs covers SC steps x NB batch = 256 free elems
            nc.tensor.matmul(
                dst, WIH[:, m, :], xsl,
                start=(r == 0), stop=False, skip_group_check=True,
            )
            nc.tensor.matmul(
                dst, BIASK[:], IND[:, m * SC * NB:(m + 1) * SC * NB],
                start=False, stop=False, skip_group_check=True,
            )

        def fill_half(t0):
            for m in range(8):
                fill_block(t0, m)

        for t in range(T):
            if t % xc == 0:
                load_xchunk(t // xc)
                load_xchunk(t // xc + 1)  # prefetch: fills read ahead of t
                xbufs.pop(t // xc - 2, None)
                if t == 0:
                    fill_half(0)
                    if T > SC:
                        fill_half(SC)
            th = t % tc_hist
            if th == 0:
                HIST = hpool.tile([128, tc_hist, 2, NB], BF16, tag="hist")

            hf = (t // SC) % 2
            s8 = t % SC
            PSH = PS2[hf]
            h_rhs = h_prev if t > 0 else ZH[:]
            for m in range(8):
                for k in range(2):
                    nc.tensor.matmul(
                        PSH[:, m // 2, m % 2, s8, :],
                        WHH[:, k, m, :], h_rhs[:, k, :],
                        start=False, stop=(k == 1), skip_group_check=True,
                    )

            # spread the next chunk's fill: one gate block per step, emitted
            # during chunk c for chunk c+1 (banks freed at end of chunk c-1);
            # sits on the PE queue right after this step's recur matmuls.
            if t >= SC and (t // SC + 2) * SC <= T:
                fill_block((t // SC + 1) * SC, s8)

            # sigmoid split: [g,g,i,i,f,f] (banks 0-2, ready after 12 MMs)
            # unblocks the DVE chain; [o,o] (bank 3) only needed at the end.
            s6 = sml.tile([128, 6, NB], F32, tag="s6")
            nc.scalar.activation(s6[:], PSH[:, 0:3, :, s8, :], AF.Sigmoid)
            so = sml.tile([128, 2, NB], F32, tag="so")
            nc.scalar.activation(so[:], PSH[:, 3, :, s8, :], AF.Sigmoid)

            vq = sml.tile([128, 2, NB], F32, tag="vq")
            nc.vector.scalar_tensor_tensor(
                vq[:], s6[:, 0:2, :], 0.5, s6[:, 2:4, :], OP.subtract, OP.mult)
            u = sml.tile([128, 2, NB], F32, tag="u")
            cp = c_prev if t > 0 else ZC[:]
            nc.vector.tensor_tensor(u[:], s6[:, 4:6, :], cp, OP.mult)
            c_new = cpool.tile([128, 2, NB], F32, tag="c")
            nc.vector.scalar_tensor_tensor(
                c_new[:], vq[:], 4.0, u[:], OP.mult, OP.add)
            sc_t = sml.tile([128, 2, NB], F32, tag="sc")
            nc.scalar.activation(sc_t[:], c_new[:], AF.Sigmoid)
            h_slot = HIST[:, th, :, :]
            nc.vector.scalar_tensor_tensor(
                h_slot, sc_t[:], 0.5, so[:], OP.subtract, OP.mult)

            c_prev = c_new[:]
            h_prev = HIST[:, th, :, :]

            if th == tc_hist - 1:
                t0 = t - (tc_hist - 1)
                nc.sync.dma_start(hist_d[:, t0:t0 + tc_hist, :, :], HIST[:])
    nc.compile()
    return nc


def _get_nc(T):
    if T not in _cache:
        _cache[T] = build(T)
    return _cache[T]


def _prep_core_inputs(x, Wih, Whh, bih, bhh, T):
    """Per weight-set host prep. Returns (shared weight arrays, xT per 4 cores).

    Gate order permuted torch [i,f,g,o] -> [g,i,f,o]; g rows scaled 2x
    (tanh-as-sigmoid trick); all Whh columns scaled 2x (h stored halved)."""
    perm = np.concatenate([np.arange(512, 768), np.arange(0, 256),
                           np.arange(256, 512), np.arange(768, 1024)])
    rowscale = np.ones((1024, 1), np.float32)
    rowscale[0:256] = 2.0   # g rows (after permutation)
    Wihp = np.asarray(Wih)[perm] * rowscale            # [1024, 128]
    Whhp = np.asarray(Whh)[perm] * (2.0 * rowscale)    # [1024, 256]
    biasp = ((np.asarray(bih) + np.asarray(bhh))[perm] * rowscale[:, 0])

    whhT = Whhp.reshape(8, 128, 2, 128).transpose(3, 2, 0, 1)  # [p,k,m,j]
    wihT = Wihp.reshape(8, 128, 128).transpose(2, 0, 1)        # [p,m,j]
    biasK = biasp.reshape(8, 128)
    whhT = whhT.astype(ml_dtypes.bfloat16)
    wihT = wihT.astype(ml_dtypes.bfloat16)
    biasK = biasK.astype(ml_dtypes.bfloat16)

    # batch-128 for this weight set: seqs 0-63 = x_real (x[0]), 64-127 = x_imag
    # x: [2, B, I, T]; per seq [I, T] slice. xT per core: [128, T, 32]
    xTs = []
    xall = np.concatenate([np.asarray(x)[0], np.asarray(x)[1]], axis=0)  # [128, I, T]
    for g in range(4):
        sl = xall[32 * g:32 * g + 32]             # [32, I, T]
        xT = sl.transpose(1, 2, 0)[:, :T, :]      # [I, T, 32]
        xTs.append(np.ascontiguousarray(xT).astype(ml_dtypes.bfloat16))
    return whhT, wihT, biasK, xTs


def _run(x, Wih_r, Whh_r, bih_r, bhh_r, Wih_i, Whh_i, bih_i, bhh_i, T,
         trace=False, tmpdir=None):
    nc = _get_nc(T)
    ind = np.kron(np.eye(8), np.ones((1, SC * NB))).astype(ml_dtypes.bfloat16)

    whhT_r, wihT_r, biasK_r, xTs_r = _prep_core_inputs(x, Wih_r, Whh_r, bih_r, bhh_r, T)
    whhT_i, wihT_i, biasK_i, _ = _prep_core_inputs(x, Wih_i, Whh_i, bih_i, bhh_i, T)
    xTs_i = xTs_r  # same input data for both weight sets

    in_maps = []
    for core in range(NCORES):
        ws = core // 4
        g = core % 4
        whhT, wihT, biasK = (whhT_r, wihT_r, biasK_r) if ws == 0 else (whhT_i, wihT_i, biasK_i)
        xT = (xTs_r if ws == 0 else xTs_i)[g]
        in_maps.append({
            "xT": xT, "whhT": whhT, "wihT": wihT, "biasK": biasK, "ind": ind,
        })
    res = run_bass_kernel_spmd(nc, in_maps, core_ids=list(range(NCORES)),
                               trace=trace, tmpdir=tmpdir)
    results = res.results

    # reassemble: hist [128, 2, 32, T] (h/2 in bf16) -> [H=256, 32, T] per core
    def hmat(ws):
        parts = []
        for g in range(4):
            h = results[4 * ws + g]["hist"].astype(np.float32)
            parts.append(h.transpose(2, 0, 3, 1).reshape(256, NB, T))
        return np.concatenate(parts, axis=1)  # [256, 128, T]

    Hr = hmat(0)
    Hi = hmat(1)
    L_r = (Hr[:, 0:64] - Hi[:, 64:128]) * 2.0   # [256, 64, T]; 2x undoes h/2
    L_i = (Hr[:, 64:128] + Hi[:, 0:64]) * 2.0
    out = np.stack([L_r.transpose(1, 0, 2), L_i.transpose(1, 0, 2)], axis=0)
    return np.ascontiguousarray(out.astype(np.float32)), res


def kernel(x, Wih_r, Whh_r, bih_r, bhh_r, Wih_i, Whh_i, bih_i, bhh_i):
    out, _ = _run(x, Wih_r, Whh_r, bih_r, bhh_r,
                  Wih_i, Whh_i, bih_i, bhh_i, T_FULL)
    return out


# revision 20
# speedup vs baseline: 1.3288x; 1.0542x over previous
"""ComplexLSTM Trainium2 kernel.

Problem: x [2, 64, 128, 1024] (real/imag, B, I, T) -> out [2, 64, 256, 1024].
Four real LSTM applications: lstm_r(x_real), lstm_r(x_imag), lstm_i(x_real),
lstm_i(x_imag); combined as L_r = r(xr) - i(xim), L_i = r(xim) + i(xr).

Sharding: 2 weight-sets x 128 sequences each = 256 independent sequences.
8 cores x 32 sequences (cores 0-3: r-weights, cores 4-7: i-weights).

Device layout (fully transposed state, weights-stationary matmuls):
  PSUM is one [128, 8, 2, 8, 32] f32 tile: bank m = gate block m (order
  [g,g,i,i,f,f,o,o]), split in two half-chunks of 8 steps x 32 batch.
  The x-projection + bias for each half-chunk are pre-accumulated into PSUM
  off the critical path (8 fat Wih matmuls free=256 + 8 indicator bias
  matmuls); the per-step work on PE is only the 16 recurrent Whh matmuls
  (lhsT = WhhT tiles bf16, rhs = h'.T slice) accumulating on top.
  Tail per step: one sigmoid over all 8 blocks (g-gate rows pre-scaled 2x on
  host so tanh(g) = 2*sig(2g)-1), then fused DVE ops on [128,2,32]:
    v_q = (s_g - 0.5) * s_i            (= i*tanh(g) / 2)
    u   = s_f * c2_prev
    c2  = 4*v_q + u                    (c2 = 2*c)
    s_c = sigmoid(c2)                  (= (tanh(c)+1)/2)
    h'  = (s_c - 0.5) * s_o            (= h / 2)
  h' written bf16 into a T-chunk history buffer that doubles as the matmul
  rhs for the next step; Whh is pre-scaled 2x to compensate h'=h/2 and the
  host combine multiplies the final output by 2.
"""

import numpy as np
import ml_dtypes
from contextlib import ExitStack

import concourse.bass as bass
import concourse.bacc as bacc
import concourse.tile as tile
from concourse import mybir
from concourse.bass_utils import run_bass_kernel_spmd

BF16 = mybir.dt.bfloat16
F32 = mybir.dt.float32
AF = mybir.ActivationFunctionType
OP = mybir.AluOpType

B, I, T_FULL, H = 64, 128, 1024, 256
NB = 32          # batch (sequences) per core
NCORES = 8
SC = 8           # steps per PSUM half-chunk
TC = 128         # history chunk (steps per output DMA)
XC = 64          # x input chunk (steps per input DMA)

_cache = {}


def build(T):
    nc = bacc.Bacc("TRN2", target_bir_lowering=False, debug=False)

    tc_hist = max(1, min(TC, T))
    xc = max(1, min(XC, T))
    assert T % tc_hist == 0 and T % xc == 0 and T % SC == 0

    xT_d = nc.declare_dram_parameter("xT", [128, T, NB], BF16, isOutput=False)
    whhT_d = nc.declare_dram_parameter("whhT", [128, 2, 8, 128], BF16, isOutput=False)
    wihT_d = nc.declare_dram_parameter("wihT", [128, 8, 128], BF16, isOutput=False)
    biasbc_d = nc.declare_dram_parameter("biasbc", [128, 4, 2, SC, NB], F32,
                                         isOutput=False)
    hist_d = nc.declare_dram_parameter("hist", [128, T, 2, NB], BF16, isOutput=True)

    with tile.TileContext(nc) as tc, ExitStack() as ctx:
        consts = ctx.enter_context(tc.tile_pool(name="consts", bufs=1))
        xin = ctx.enter_context(tc.tile_pool(name="xin", bufs=2))
        hpool = ctx.enter_context(tc.tile_pool(name="hist", bufs=2))
        psum = ctx.enter_context(tc.tile_pool(name="psum", bufs=1, space="PSUM"))
        sml = ctx.enter_context(tc.tile_pool(name="small", bufs=3))
        cpool = ctx.enter_context(tc.tile_pool(name="cpool", bufs=3))

        WHH = consts.tile([128, 2, 8, 128], BF16)
        nc.sync.dma_start(WHH[:], whhT_d[:])
        WIH = consts.tile([128, 8, 128], BF16)
        nc.sync.dma_start(WIH[:], wihT_d[:])
        BIASBC = consts.tile([128, 4, 2, SC, NB], F32)
        nc.sync.dma_start(BIASBC[:], biasbc_d[:])
        ZH = consts.tile([128, 2, NB], BF16)
        nc.vector.memset(ZH[:], 0.0)
        ZC = consts.tile([128, 2, NB], F32)
        nc.vector.memset(ZC[:], 0.0)

        # All of PSUM: [partition, bank-in-group, block-in-bank,
        # step-in-chunk, batch] x 2 parities. Chunk parity p uses banks
        # 4p..4p+3; bank q holds gate blocks 2q and 2q+1. start=True
        # (bank-granular reset, ZERO_REGION=2KB) is issued only on the first
        # matmul into each bank per chunk. Two separate tiles so the Tile
        # framework's per-tile dependency tracking doesn't serialize the
        # next chunk's fill behind the current chunk's sigmoid reads.
        # gif tile = banks 0-2 (gate blocks 0-5), o tile = bank 3 (blocks
        # 6-7), per parity. Separate tiles so sigma_gif's read dep covers
        # only the 12 gif recur matmuls (dep tracking is per-tile).
        PSGA = psum.tile([128, 3, 2, SC, NB], F32, tag="psga")
        PSOA = psum.tile([128, 1, 2, SC, NB], F32, tag="psoa")
        PSGB = psum.tile([128, 3, 2, SC, NB], F32, tag="psgb")
        PSOB = psum.tile([128, 1, 2, SC, NB], F32, tag="psob")
        PSG2 = [PSGA, PSGB]
        PSO2 = [PSOA, PSOB]

        HIST = None
        c_prev = None
        h_prev = None  # AP into HIST for h'.T(t-1)
        nxc = T // xc
        xbufs = {}  # x chunk index -> SBUF tile (bufs=2 pool keeps 2 live)

        def load_xchunk(c):
            if c < nxc and c not in xbufs:
                xb = xin.tile([128, xc, NB], BF16, tag="xbuf")
                nc.sync.dma_start(xb[:], xT_d[:, c * xc:(c + 1) * xc, :])
                xbufs[c] = xb

        def psum_dst(hf, m):
            q, r = m // 2, m % 2
            if q < 3:
                return PSG2[hf][:, q, r, :, :]
            return PSO2[hf][:, 0, r, :, :]

        def fill_block(t0, m):
            """Emit the xproj matmul for gate block m of the chunk at steps
            t0..t0+SC-1 (parity (t0//SC)%2). Off the critical path: runs on
            PE during tails. Blocks must be emitted in order (start=True on
            the even block resets the whole bank)."""
            hf = (t0 // SC) % 2
            xb = xbufs[t0 // xc]
            xsl = xb[:, t0 % xc:t0 % xc + SC, :]
            # rhs covers SC steps x NB batch = 256 free elems
            nc.tensor.matmul(
                psum_dst(hf, m), WIH[:, m, :], xsl,
                start=(m % 2 == 0), stop=False, skip_group_check=True,
            )

        def fill_bias(t0, which=2):
            """One DVE add per chunk folds the (broadcast) gate bias into
            the freshly x-projected PSUM chunk -- replaces 8 fat bias
            matmuls on the PE."""
            hf = (t0 // SC) % 2
            if which in (0, 2):
                nc.vector.tensor_tensor(
                    PSG2[hf][:], PSG2[hf][:], BIASBC[:, 0:3, :, :, :], OP.add)
            if which in (1, 2):
                nc.vector.tensor_tensor(
                    PSO2[hf][:], PSO2[hf][:], BIASBC[:, 3, :, :, :], OP.add)

        def fill_half(t0):
            for m in range(8):
                fill_block(t0, m)
            fill_bias(t0)

        for t in range(T):
            if t % xc == 0:
                load_xchunk(t // xc)
                load_xchunk(t // xc + 1)  # prefetch: fills read ahead of t
                xbufs.pop(t // xc - 2, None)
                if t == 0:
                    fill_half(0)
                    if T > SC:
                        fill_half(SC)
            th = t % tc_hist
            if th == 0:
                HIST = hpool.tile([128, tc_hist, 2, NB], BF16, tag="hist")

            hf = (t // SC) % 2
            s8 = t % SC
            h_rhs = h_prev if t > 0 else ZH[:]
            for m in range(8):
                for k in range(2):
                    nc.tensor.matmul(
                        psum_dst(hf, m)[:, s8, :],
                        WHH[:, k, m, :], h_rhs[:, k, :],
                        start=False, stop=(k == 1), skip_group_check=True,
                    )

            # spread the next chunk's fill: one gate block per step, emitted
            # during chunk c for chunk c+1 (banks freed at end of chunk c-1);
            # sits on the PE queue right after this step's recur matmuls.
            if t >= SC and (t // SC + 2) * SC <= T:
                fill_block((t // SC + 1) * SC, s8)

            # sigmoid split: [g,g,i,i,f,f] (banks 0-2, ready after 12 MMs)
            # unblocks the DVE chain; [o,o] (bank 3) only needed at the end.
            s6 = sml.tile([128, 6, NB], F32, tag="s6")
            nc.scalar.activation(s6[:], PSG2[hf][:, :, :, s8, :], AF.Sigmoid)
            so = sml.tile([128, 2, NB], F32, tag="so")
            nc.scalar.activation(so[:], PSO2[hf][:, 0, :, s8, :], AF.Sigmoid)

            vq = sml.tile([128, 2, NB], F32, tag="vq")
            nc.vector.scalar_tensor_tensor(
                vq[:], s6[:, 0:2, :], 0.5, s6[:, 2:4, :], OP.subtract, OP.mult)
            u = sml.tile([128, 2, NB], F32, tag="u")
            cp = c_prev if t > 0 else ZC[:]
            nc.vector.tensor_tensor(u[:], s6[:, 4:6, :], cp, OP.mult)
            c_new = cpool.tile([128, 2, NB], F32, tag="c")
            nc.vector.scalar_tensor_tensor(
                c_new[:], vq[:], 4.0, u[:], OP.mult, OP.add)
            sc_t = sml.tile([128, 2, NB], F32, tag="sc")
            nc.scalar.activation(sc_t[:], c_new[:], AF.Sigmoid)
            h_slot = HIST[:, th, :, :]
            nc.vector.scalar_tensor_tensor(
                h_slot, sc_t[:], 0.5, so[:], OP.subtract, OP.mult)

            c_prev = c_new[:]
            h_prev = HIST[:, th, :, :]

            # bias-add for the next chunk once its xproj matmuls are in
            # (emitted after this step's tail; split across the last two
            # steps of the chunk to limit DVE-queue delay)
            if t >= SC and (t // SC + 2) * SC <= T:
                if s8 == SC - 2:
                    fill_bias((t // SC + 1) * SC, which=0)  # needs blocks 0-5
                elif s8 == SC - 1:
                    fill_bias((t // SC + 1) * SC, which=1)  # needs blocks 6-7

            if th == tc_hist - 1:
                t0 = t - (tc_hist - 1)
                nc.sync.dma_start(hist_d[:, t0:t0 + tc_hist, :, :], HIST[:])
    nc.compile()
    return nc


def _get_nc(T):
    if T not in _cache:
        _cache[T] = build(T)
    return _cache[T]


def _prep_core_inputs(x, Wih, Whh, bih, bhh, T):
    """Per weight-set host prep. Returns (shared weight arrays, xT per 4 cores).

    Gate order permuted torch [i,f,g,o] -> [g,i,f,o]; g rows scaled 2x
    (tanh-as-sigmoid trick); all Whh columns scaled 2x (h stored halved)."""
    perm = np.concatenate([np.arange(512, 768), np.arange(0, 256),
                           np.arange(256, 512), np.arange(768, 1024)])
    rowscale = np.ones((1024, 1), np.float32)
    rowscale[0:256] = 2.0   # g rows (after permutation)
    Wihp = np.asarray(Wih)[perm] * rowscale            # [1024, 128]
    Whhp = np.asarray(Whh)[perm] * (2.0 * rowscale)    # [1024, 256]
    biasp = ((np.asarray(bih) + np.asarray(bhh))[perm] * rowscale[:, 0])

    whhT = Whhp.reshape(8, 128, 2, 128).transpose(3, 2, 0, 1)  # [p,k,m,j]
    wihT = Wihp.reshape(8, 128, 128).transpose(2, 0, 1)        # [p,m,j]
    whhT = whhT.astype(ml_dtypes.bfloat16)
    wihT = wihT.astype(ml_dtypes.bfloat16)
    # broadcast bias [p, q, r, SC, NB] f32 (m = 2q + r)
    bias_pm = biasp.reshape(8, 128).T.reshape(128, 4, 2)  # [p, q, r]
    biasbc = np.ascontiguousarray(np.broadcast_to(
        bias_pm[:, :, :, None, None].astype(np.float32),
        (128, 4, 2, SC, NB)))

    # batch-128 for this weight set: seqs 0-63 = x_real (x[0]), 64-127 = x_imag
    # x: [2, B, I, T]; per seq [I, T] slice. xT per core: [128, T, 32]
    xTs = []
    xall = np.concatenate([np.asarray(x)[0], np.asarray(x)[1]], axis=0)  # [128, I, T]
    for g in range(4):
        sl = xall[32 * g:32 * g + 32]             # [32, I, T]
        xT = sl.transpose(1, 2, 0)[:, :T, :]      # [I, T, 32]
        xTs.append(np.ascontiguousarray(xT).astype(ml_dtypes.bfloat16))
    return whhT, wihT, biasbc, xTs


def _run(x, Wih_r, Whh_r, bih_r, bhh_r, Wih_i, Whh_i, bih_i, bhh_i, T,
         trace=False, tmpdir=None):
    nc = _get_nc(T)

    whhT_r, wihT_r, biasbc_r, xTs_r = _prep_core_inputs(x, Wih_r, Whh_r, bih_r, bhh_r, T)
    whhT_i, wihT_i, biasbc_i, _ = _prep_core_inputs(x, Wih_i, Whh_i, bih_i, bhh_i, T)
    xTs_i = xTs_r  # same input data for both weight sets

    in_maps = []
    for core in range(NCORES):
        ws = core // 4
        g = core % 4
        whhT, wihT, biasbc = (whhT_r, wihT_r, biasbc_r) if ws == 0 else (whhT_i, wihT_i, biasbc_i)
        xT = (xTs_r if ws == 0 else xTs_i)[g]
        in_maps.append({
            "xT": xT, "whhT": whhT, "wihT": wihT, "biasbc": biasbc,
        })
    res = run_bass_kernel_spmd(nc, in_maps, core_ids=list(range(NCORES)),
                               trace=trace, tmpdir=tmpdir)
    results = res.results

    # reassemble: hist [128, 2, 32, T] (h/2 in bf16) -> [H=256, 32, T] per core
    def hmat(ws):
        parts = []
        for g in range(4):
            h = results[4 * ws + g]["hist"].astype(np.float32)
            parts.append(h.transpose(2, 0, 3, 1).reshape(256, NB, T))
        return np.concatenate(parts, axis=1)  # [256, 128, T]

    Hr = hmat(0)
    Hi = hmat(1)
    L_r = (Hr[:, 0:64] - Hi[:, 64:128]) * 2.0   # [256, 64, T]; 2x undoes h/2
    L_i = (Hr[:, 64:128] + Hi[:, 0:64]) * 2.0
    out = np.stack([L_r.transpose(1, 0, 2), L_i.transpose(1, 0, 2)], axis=0)
    return np.ascontiguousarray(out.astype(np.float32)), res


def kernel(x, Wih_r, Whh_r, bih_r, bhh_r, Wih_i, Whh_i, bih_i, bhh_i):
    out, _ = _run(x, Wih_r, Whh_r, bih_r, bhh_r,
                  Wih_i, Whh_i, bih_i, bhh_i, T_FULL)
    return out
